# revision 23
# baseline (speedup 1.0000x reference)
"""MultiHeadAttention (cosine-sim, no softmax) + residual + LayerNorm on 8 TRN2 cores.

Reference math (per sample row x of q/k/v, D=2048, H=16, HD=128):
  qp = q @ Wq + bq   (kept as residual)
  kp = k @ Wk + bk ; vp = v @ Wv + bv
  per head h: qn = qh/||qh||, kn = kh/||kh||
  s[h,g] = (qn_h . kn_g) / HD          # [16,16] per sample
  o[h] = sum_g s[h,g] * vh_g           # [16,128]
  o_flat[hd*16+h] = o[h,hd]            # interleaved flatten
  o2 = o_flat @ Wo + bo
  x = qp + o2 ; out = layernorm(x) * gamma + beta
Sharding: pure data-parallel over batch (4096 rows/core), weights replicated.

Device strategy (per core):
  - The q projection (the residual path, accuracy-critical) runs in bf16.
    The k/v/o projections run in fp8 e4m3 with perf_mode=DoubleRow (2
    contraction rows per PE cell -> half the matmul instructions).  Their
    error feeds only the attention output o2, which is ~0.25% of |x|, so
    fp8's ~3% error contributes ~1e-4 to the final relative error.
  - Scale folding: q is host-scaled by 2^13 and Wk/Wv/Wo by 2^5 so every
    fp8 operand sits near unit RMS; the score mask carries 1/16 so the
    attention output o lands at RMS ~0.6 for its fp8 requantization, and
    the o2 PSUM comes out at exactly 2^13*o2 -- matching the 2^13-scaled
    qp residual with NO descale op (LayerNorm is scale-invariant; eps is
    scaled by 2^26 to stay faithful).  k-side scales cancel in normalize.
  - Per-head norms: one ACT Square [128,512] + one segmented DVE
    tensor_reduce + one ACT Rsqrt per evac (the accum_out path costs a
    334ns ACTIVATION_READ_ACCUMULATOR per head -- 4x more ACT time).
  - PSUM evacuations ride ACT (scalar.copy beats DVE tensor_copy on
    PSUM reads); transpose evacs alternate DVE/ACT to balance.
  - The PE instruction stream is software-pipelined: each n-group's
    normalize+transpose work is emitted one group late, attention o
    matmuls one group late, and the NEXT chunk's v projection matmuls
    are interleaved into the attention groups so the PE never idles
    long enough for the HAM activity monitor to re-throttle the clock.
  - Attention evacs batched 2 blocks per op in shared PSUM tiles.
  - NB: DVE tensor_tensor_reduce with in0==in1 crashes TRN2 HW (passes
    CoreSim); norms use the ACT Square + DVE reduce path instead.
"""

from contextlib import ExitStack

import numpy as np
import ml_dtypes

import concourse.bass as bass
import concourse.bacc as bacc
import concourse.mybir as mybir
import concourse.tile as tile
from concourse.bass_utils import run_bass_kernel_spmd

BF16 = mybir.dt.bfloat16
F8 = mybir.dt.float8e4
F32 = mybir.dt.float32
DR = mybir.MatmulPerfMode.DoubleRow
AF = mybir.ActivationFunctionType

B, D, H, HD = 32768, 2048, 16, 128
NCORES = 8
EPS = 1e-5
CHUNK = 512          # samples per chunk (8 chunks per core at BL=4096)
NG = D // 512        # 4 n-chunks of 512 columns
KO = D // 128        # 16 contraction blocks
KD = KO // 2         # 8 DoubleRow contraction pairs
SBLK = 8             # samples per attention block matmul (8*16 = 128)
NBT = CHUNK // 128   # 4 b-tiles per chunk
NBLK = CHUNK // SBLK # 64 attention blocks per chunk
AGRP = 8             # attention blocks per pipelined group

QSC = 8192.0         # host scale on q (2^13)
WSC = 32.0           # host scale on Wk/Wv/Wo (2^5)
MSC = 16.0           # score mask divisor: 1/MSC instead of 1/HD so that
                     # o_psum = (WSC/MSC)*128*o = 256*o  (fp8 sweet spot)
                     # and o2_psum = 256*WSC*o2 = QSC*o2 (matches QSC*qp)


def build_bass(bl, repeat=1, affine=False):
    """Build the per-core Bass program for bl batch rows (bl % CHUNK == 0)."""
    nc = bacc.Bacc()

    nch = bl // CHUNK
    # inputs/weights host-packed so every DMA slab is contiguous per
    # partition row (8-16KB descriptors instead of 512B strided rows)
    qT = nc.dram_tensor("qT", [128, nch, KO, CHUNK], BF16,
                        kind="ExternalInput")
    kT = nc.dram_tensor("kT", [128, nch, KO, CHUNK], F8,
                        kind="ExternalInput")
    vT = nc.dram_tensor("vT", [128, nch, KO, CHUNK], F8,
                        kind="ExternalInput")
    Wq = nc.dram_tensor("Wq", [128, NG, KO, 512], BF16,
                        kind="ExternalInput")
    Wk = nc.dram_tensor("Wk", [128, NG, KO, 512], F8, kind="ExternalInput")
    Wv = nc.dram_tensor("Wv", [D, D], F8, kind="ExternalInput")  # g-major
    Wo = nc.dram_tensor("Wo", [128, NG, KO, 512], F8, kind="ExternalInput")
    bv = nc.dram_tensor("bv", [128, KO], F32, kind="ExternalInput")
    if affine:
        bq = nc.dram_tensor("bq", [1, D], BF16, kind="ExternalInput")
        bk = nc.dram_tensor("bk", [1, D], BF16, kind="ExternalInput")
        bo = nc.dram_tensor("bo", [1, D], BF16, kind="ExternalInput")
        gamma = nc.dram_tensor("gamma", [1, D], BF16, kind="ExternalInput")
        beta = nc.dram_tensor("beta", [1, D], BF16, kind="ExternalInput")
    ident = nc.dram_tensor("ident", [128, 128], BF16, kind="ExternalInput")
    identf8 = nc.dram_tensor("identf8", [128, 128], F8,
                             kind="ExternalInput")
    mask = nc.dram_tensor("mask", [128, 256], BF16, kind="ExternalInput")
    ones = nc.dram_tensor("ones", [1, 128], BF16, kind="ExternalInput")
    out = nc.dram_tensor("out", [bl, D], F32, kind="ExternalOutput")

    nchunks = bl // CHUNK

    with tile.TileContext(nc) as tc, ExitStack() as ctx:
        consts = ctx.enter_context(tc.tile_pool(name="consts", bufs=1))
        q_pool = ctx.enter_context(tc.tile_pool(name="qbuf", bufs=1))
        kv_pool = ctx.enter_context(tc.tile_pool(name="kvbuf", bufs=2))
        ws_pool = ctx.enter_context(
            tc.tile_pool(name="ws", bufs=1 if affine else 2))
        ws8_pool = ctx.enter_context(tc.tile_pool(name="ws8", bufs=2))
        wsv_pool = ctx.enter_context(tc.tile_pool(name="wsv", bufs=4))
        chunk_pool = ctx.enter_context(tc.tile_pool(name="chunkbuf", bufs=1))
        vhT_pool = ctx.enter_context(tc.tile_pool(name="vhTbuf", bufs=2))
        kp_pool = ctx.enter_context(tc.tile_pool(name="kp", bufs=6))
        nrm_pool = ctx.enter_context(tc.tile_pool(name="nrm", bufs=2))
        st_pool = ctx.enter_context(tc.tile_pool(name="stvb", bufs=AGRP))
        scr_pool = ctx.enter_context(tc.tile_pool(name="scr", bufs=2))
        small_pool = ctx.enter_context(tc.tile_pool(name="small", bufs=4))
        out_pool = ctx.enter_context(tc.tile_pool(name="outb", bufs=4))
        proj_psum = ctx.enter_context(
            tc.tile_pool(name="proj_psum", bufs=5, space="PSUM"))
        att_psum = ctx.enter_context(
            tc.tile_pool(name="att_psum", bufs=3, space="PSUM"))

        # ---- constants ----
        ident_sb = consts.tile([128, 128], BF16)
        nc.sync.dma_start(out=ident_sb, in_=ident[:, :])
        ident8_sb = consts.tile([128, 128], F8)
        nc.sync.dma_start(out=ident8_sb, in_=identf8[:, :])
        mask_sb = consts.tile([128, 2, 128], BF16)
        nc.sync.dma_start(
            out=mask_sb, in_=mask[:, :].rearrange("p (b m) -> p b m", b=2))
        ones_sb = consts.tile([1, 128], BF16)
        nc.sync.dma_start(out=ones_sb, in_=ones[:, :])
        bv_sb = consts.tile([128, KO], F32)
        nc.sync.dma_start(out=bv_sb, in_=bv[:, :])
        eps_sb = consts.tile([128, 1], F32)
        nc.vector.memset(eps_sb, EPS * QSC * QSC)
        if affine:
            bq_sb = consts.tile([1, D], BF16)
            nc.sync.dma_start(out=bq_sb, in_=bq[:, :])
            bk_sb = consts.tile([1, D], BF16)
            nc.sync.dma_start(out=bk_sb, in_=bk[:, :])
            bo_sb = consts.tile([1, D], BF16)
            nc.sync.dma_start(out=bo_sb, in_=bo[:, :])
            g_ap = gamma[:, :]
            gamma_sb = consts.tile([128, D], BF16)
            nc.sync.dma_start(
                out=gamma_sb,
                in_=bass.AP(tensor=g_ap.tensor, offset=g_ap.offset,
                            ap=[[0, 128], [1, D]]))
            b_ap = beta[:, :]
            beta_sb = consts.tile([128, D], BF16)
            nc.sync.dma_start(
                out=beta_sb,
                in_=bass.AP(tensor=b_ap.tensor, offset=b_ap.offset,
                            ap=[[0, 128], [1, D]]))


        for _rep in range(repeat):

            # one head-slab of the v projection: 8 DoubleRow matmuls + evac
            def emit_v_group(vT_sb, vhT_sb, g, c):
                ws_v = wsv_pool.tile([128, D], F8, tag="wsv")
                nc.sync.dma_start(out=ws_v,
                                  in_=Wv[g * 128:(g + 1) * 128, :])
                ws_v3 = ws_v[:, :].rearrange("p (ko m) -> p ko m", m=128)
                ps_v = proj_psum.tile([128, CHUNK], F32, tag="pp",
                                      name=f"pv_{_rep}_{c}_{g}")
                for kd in range(KD):
                    nc.tensor.matmul(
                        ps_v,
                        ws_v3[:, 2 * kd:2 * kd + 2, :],
                        vT_sb[:, 2 * kd:2 * kd + 2, :],
                        start=(kd == 0), stop=(kd == KD - 1),
                        perf_mode=DR)
                # add per-partition bias while copying psum->sbuf bf16
                nc.scalar.activation(
                    out=vhT_sb[:, :, g, :],
                    in_=ps_v[:, :].rearrange("p (blk s) -> p blk s", s=SBLK),
                    func=AF.Identity, bias=bv_sb[:, g:g + 1], scale=1.0)

            vT_next = kv_pool.tile([128, KO, CHUNK], F8, tag="kv",
                                   name=f"vT_{_rep}_0")
            nc.sync.dma_start(out=vT_next, in_=vT[:, 0])
            vhT_cur = None

            for c in range(nchunks):
              b0 = c * CHUNK
              vT_sb = vT_next
              if c == 0:
                  vhT_cur = vhT_pool.tile([128, NBLK, H, SBLK], F8,
                                          tag="vhT", name=f"vhT_{_rep}_0")
                  for g in range(H):
                      emit_v_group(vT_sb, vhT_cur, g, c)
              vhT_sb = vhT_cur
              qT_sb = q_pool.tile([128, KO, CHUNK], BF16, tag="qT")
              nc.sync.dma_start(out=qT_sb, in_=qT[:, c])

              # chunk-lifetime buffers (interleaved [hd,blk,h,s]; b=blk*8+s)
              qp_sb = chunk_pool.tile([128, NBT, D], BF16, tag="qp",
                                      bufs=2)
              qnT_sb = chunk_pool.tile([128, NBLK, H, SBLK], F8, tag="qnT")
              knT_sb = chunk_pool.tile([128, NBLK, H, SBLK], F8, tag="knT")
              oT_sb = chunk_pool.tile([128, H, CHUNK], F8, tag="oT")

              # ---- q then k projections (natural) + normalize + transpose --
              # PASS1 (matmuls+evac) runs one n-group ahead of PASS2
              # (normalize+transposes) so the PE queue never stalls.
              def normalize_group(ng, evacs, dstT):
                for bt, xp in evacs:
                    scr = scr_pool.tile([128, 512], BF16, tag="scr")
                    nc.scalar.activation(out=scr, in_=xp, func=AF.Square)
                    rr = small_pool.tile([128, 4], F32, tag="rr")
                    nc.vector.tensor_reduce(
                        out=rr,
                        in_=scr[:, :].rearrange("p (h m) -> p h m", m=128),
                        axis=mybir.AxisListType.X, op=mybir.AluOpType.add)
                    nc.scalar.activation(out=rr, in_=rr, func=AF.Sqrt)
                    nc.vector.reciprocal(out=rr, in_=rr)
                    nrm = nrm_pool.tile([128, 512], BF16, tag="nrm")
                    for h4 in range(4):
                        nc.vector.tensor_scalar_mul(
                            out=nrm[:, h4 * 128:(h4 + 1) * 128],
                            in0=xp[:, h4 * 128:(h4 + 1) * 128],
                            scalar1=rr[:, h4:h4 + 1])
                    for h4 in range(4):
                        # normal-mode matmul X.T@I transposes ~2x faster
                        # than transpose_mode (no PE_SBUF latency stall,
                        # counts as HAM-warm activity)
                        tp = att_psum.tile([128, 128], F32, tag="ap")
                        nc.tensor.matmul(
                            tp, nrm[:, h4 * 128:(h4 + 1) * 128], ident_sb,
                            start=True, stop=True)
                        dst = dstT[:, bt * 16:(bt + 1) * 16, ng * 4 + h4, :]
                        src = tp[:, :].rearrange("p (blk s) -> p blk s",
                                                 s=SBLK)
                        if h4 % 2 == 0:
                            nc.vector.tensor_copy(out=dst, in_=src)
                        else:
                            nc.scalar.copy(out=dst, in_=src)

              def proj_qk(xT_sb, W3, b_sb, is_q, tail_filler=None,
                          head_filler=None, defer_final=False):
                dstT = qnT_sb if is_q else knT_sb
                ws_tiles = {}

                def fetch(ng):
                    # one-group weight-slab lookahead; Wq rides the sync
                    # ring, Wk the vector ring (parallel queues)
                    if ng < NG and ng not in ws_tiles:
                        if is_q:
                            t = ws_pool.tile([128, KO, 512], BF16, tag="ws")
                            nc.sync.dma_start(out=t, in_=W3[:, ng])
                        else:
                            t = ws8_pool.tile([128, KO, 512], F8, tag="ws8")
                            nc.sync.dma_start(out=t, in_=W3[:, ng])
                        ws_tiles[ng] = t

                fetch(0)
                fetch(1)
                pending = None
                for ng in range(NG):
                    n0 = ng * 512
                    ws = ws_tiles[ng]
                    all_ps = []
                    for half in range(2):
                        bts = (2 * half, 2 * half + 1)
                        ps_pair = [
                            proj_psum.tile([128, 512], F32, tag="pp",
                                           name=f"pp_{c}_{is_q}_{ng}_{bt}")
                            for bt in bts]
                        if is_q:
                            for ko in range(KO):
                                for i, bt in enumerate(bts):
                                    nc.tensor.matmul(
                                        ps_pair[i],
                                        xT_sb[:, ko, bt * 128:(bt + 1) * 128],
                                        ws[:, ko, :], start=(ko == 0),
                                        stop=(not affine and ko == KO - 1))
                        else:
                            for kd in range(KD):
                                for i, bt in enumerate(bts):
                                    nc.tensor.matmul(
                                        ps_pair[i],
                                        xT_sb[:, 2 * kd:2 * kd + 2,
                                              bt * 128:(bt + 1) * 128],
                                        ws[:, 2 * kd:2 * kd + 2, :],
                                        start=(kd == 0),
                                        stop=(not affine and kd == KD - 1),
                                        perf_mode=DR)
                        for i, bt in enumerate(bts):
                            if affine:
                                nc.tensor.matmul(ps_pair[i], ones_sb,
                                                 b_sb[:, n0:n0 + 512],
                                                 start=False, stop=True)
                            all_ps.append((bt, ps_pair[i]))
                    fetch(ng + 1)
                    # the caller's deferred chain (q's last normalize) goes
                    # in front so it overlaps our first two matmul groups
                    if ng == 1 and head_filler is not None:
                        head_filler()
                    # normalize of the PREVIOUS group goes on the engine
                    # queues BEFORE this group's evacs: its Square/mul chain
                    # is ready now, while these evacs must wait for the
                    # matmuls above -- order swapped = ACT head-of-line block
                    if pending is not None:
                        normalize_group(*pending)
                    evacs = []
                    for bt, ps in all_ps:
                        if is_q:
                            xp = qp_sb[:, bt, n0:n0 + 512]
                        else:
                            kp = kp_pool.tile([128, 512], BF16, tag="kp")
                            xp = kp[:, :]
                        nc.scalar.copy(out=xp, in_=ps)
                        evacs.append((bt, xp))
                    pending = (ng, evacs, dstT)
                if defer_final:
                    # the final group's normalize is emitted by the caller
                    # inside the NEXT phase's matmul stream (the in-order PE
                    # queue would otherwise stall on its transposes)
                    final = pending
                    return lambda: normalize_group(*final)
                if tail_filler is not None:
                    tail_filler()
                normalize_group(*pending)
                return None

              kT_sb = kv_pool.tile([128, KO, CHUNK], F8, tag="kv",
                                 name=f"kT_{c}")
              nc.sync.dma_start(out=kT_sb, in_=kT[:, c])
              q_final = proj_qk(qT_sb, Wq, bq_sb if affine else None, True,
                                defer_final=True)

              # next chunk's v inputs; the first 4 of its 16 projection
              # head-slabs are emitted as the k-projection tail filler (they
              # keep the PE fed while the last k normalize chain drains),
              # the remaining 12 are interleaved into the attention groups
              if c + 1 < nchunks:
                  vT_next = kv_pool.tile([128, KO, CHUNK], F8, tag="kv",
                                         name=f"vT_{_rep}_{c + 1}")
                  nc.sync.dma_start(out=vT_next, in_=vT[:, c + 1])
                  vhT_next = vhT_pool.tile([128, NBLK, H, SBLK], F8,
                                           tag="vhT",
                                           name=f"vhT_{_rep}_{c + 1}")

                  def k_tail():
                      for g in range(6):
                          emit_v_group(vT_next, vhT_next, g, c + 1)
              else:
                  vhT_next = None
                  k_tail = None

              proj_qk(kT_sb, Wk, bk_sb if affine else None, False,
                      tail_filler=k_tail, head_filler=q_final)

              # prefetch the first two Wo slabs (vector ring) so the output
              # projection never waits on weight DMA; they reuse ws8 slots
              # whose Wk readers completed during the k projection
              wo_tiles = {}

              def fetch_wo(ng):
                  if ng < NG and ng not in wo_tiles:
                      t = ws8_pool.tile([128, KO, 512], F8, tag="ws8")
                      nc.sync.dma_start(out=t, in_=Wo[:, ng])
                      wo_tiles[ng] = t

              fetch_wo(0)
              fetch_wo(1)

              # ---- attention: scores + o, 2x8 samples per psum tile ----
              def att_scores(grp):
                outs = []
                for half in range(AGRP // 2):
                    blk0 = grp * AGRP + 2 * half
                    st_ps = att_psum.tile([128, 2, 128], F32, tag="ap")
                    vb_ps = att_psum.tile([128, 2, 128], F32, tag="ap")
                    for j in range(2):
                        nc.tensor.matmul(
                            st_ps[:, j],
                            knT_sb[:, blk0 + j].rearrange("p h s -> p (h s)"),
                            qnT_sb[:, blk0 + j].rearrange("p h s -> p (h s)"),
                            start=True, stop=True)
                        nc.tensor.matmul(
                            vb_ps[:, j],
                            vhT_sb[:, blk0 + j].rearrange("p h s -> p (h s)"),
                            ident8_sb, start=True, stop=True)
                    st_sb = st_pool.tile([128, 2, 128], BF16, tag="st")
                    nc.vector.tensor_mul(out=st_sb, in0=st_ps, in1=mask_sb)
                    vb_sb = st_pool.tile([128, 2, 128], BF16, tag="vb")
                    nc.scalar.copy(out=vb_sb, in_=vb_ps)
                    outs.append((blk0, st_sb, vb_sb))
                return outs

              def att_o(outs):
                for blk0, st_sb, vb_sb in outs:
                    o_ps = att_psum.tile([128, 2, 128], F32, tag="ap")
                    for j in range(2):
                        nc.tensor.matmul(o_ps[:, j], vb_sb[:, j],
                                         st_sb[:, j], start=True, stop=True)
                    s0 = blk0 * SBLK
                    nc.scalar.copy(
                        out=oT_sb[:, :, s0:s0 + 2 * SBLK].rearrange(
                            "p h (b s) -> p b h s", b=2),
                        in_=o_ps[:, :, :].rearrange("p b (h s) -> p b h s",
                                                    h=H))

              prev = None
              for grp in range(NBLK // AGRP):
                cur = att_scores(grp)
                if vhT_next is not None and 2 <= grp < 7:
                    emit_v_group(vT_next, vhT_next, 2 + 2 * grp, c + 1)
                    emit_v_group(vT_next, vhT_next, 3 + 2 * grp, c + 1)
                if prev is not None:
                    att_o(prev)
                prev = cur
              att_o(prev)
              vhT_cur = vhT_next

              # ---- output projection + residual ----
              for ng in range(NG):
                n0 = ng * 512
                wo_s = wo_tiles[ng]
                for half in range(2):
                    bts = (2 * half, 2 * half + 1)
                    ps_pair = [proj_psum.tile([128, 512], F32, tag="pp",
                                              name=f"po_{c}_{ng}_{half}_{i}")
                               for i in range(2)]
                    for hj in range(KD):
                        for i, bt in enumerate(bts):
                            nc.tensor.matmul(
                                ps_pair[i],
                                oT_sb[:, 2 * hj:2 * hj + 2,
                                      bt * 128:(bt + 1) * 128],
                                wo_s[:, 2 * hj:2 * hj + 2, :],
                                start=(hj == 0),
                                stop=(not affine and hj == KD - 1),
                                perf_mode=DR)
                    for i, bt in enumerate(bts):
                        ps = ps_pair[i]
                        if affine:
                            nc.tensor.matmul(ps, ones_sb,
                                             bo_sb[:, n0:n0 + 512],
                                             start=False, stop=True)
                        # x = qp + o2 (in place into qp_sb)
                        nc.vector.tensor_add(
                            out=qp_sb[:, bt, n0:n0 + 512],
                            in0=qp_sb[:, bt, n0:n0 + 512], in1=ps)
                fetch_wo(ng + 2)

              # ---- layernorm + store ----
              for bt in range(NBT):
                x_ap = qp_sb[:, bt, :]
                stats = small_pool.tile([128, 4, 6], F32, tag="bn")
                for sg in range(4):
                    nc.vector.bn_stats(out=stats[:, sg, :],
                                       in_=x_ap[:, sg * 512:(sg + 1) * 512])
                mv = small_pool.tile([128, 2], F32, tag="mv")
                nc.vector.bn_aggr(out=mv, in_=stats)
                rstd = small_pool.tile([128, 1], F32, tag="rstd")
                nc.scalar.activation(out=rstd, in_=mv[:, 1:2],
                                     func=AF.Sqrt, bias=eps_sb, scale=1.0)
                nc.vector.reciprocal(out=rstd, in_=rstd)
                for ng in range(NG):
                    n0 = ng * 512
                    ot = out_pool.tile([128, 512], F32, tag="ot")
                    # (x - mu) * rstd
                    nc.vector.tensor_scalar(
                        out=ot, in0=x_ap[:, n0:n0 + 512],
                        scalar1=mv[:, 0:1], scalar2=rstd,
                        op0=mybir.AluOpType.subtract,
                        op1=mybir.AluOpType.mult)
                    if affine:
                        nc.vector.tensor_mul(out=ot, in0=ot,
                                             in1=gamma_sb[:, n0:n0 + 512])
                        nc.gpsimd.tensor_add(out=ot, in0=ot,
                                             in1=beta_sb[:, n0:n0 + 512])
                    # SWDGE queue: keeps LN-gated stores off the HWDGE
                    # rings so they never block evacs or input loads
                    nc.gpsimd.dma_start(
                        out=out[b0 + bt * 128:b0 + (bt + 1) * 128,
                                n0:n0 + 512],
                        in_=ot)

    nc.compile()
    return nc


def _pack_w(W):
    # [D, D] -> [128, NG, KO, 512] with W_h[p, ng, ko, m] = W[ko*128+p, ...]
    return np.ascontiguousarray(
        W.reshape(KO, 128, NG, 512).transpose(1, 2, 0, 3))


def _pack_x(xT, bl):
    # [D, bl] -> [128, nch, KO, CHUNK]; xT_h[p, c, ko, s] = xT[ko*128+p, ...]
    return np.ascontiguousarray(
        xT.reshape(KO, 128, bl // CHUNK, CHUNK).transpose(1, 2, 0, 3))


def _prep_host_inputs(q, k, v, Wq, bq, Wk, bk, Wv, bv, Wo, bo, gamma, beta):
    bf = ml_dtypes.bfloat16
    f8 = ml_dtypes.float8_e4m3
    qT = (q.T * QSC).astype(bf)
    kT = k.T.astype(f8)
    vT = v.T.astype(f8)
    # Wo' row h*128+hd  <- Wo row hd*16+h
    hh, dd = np.divmod(np.arange(D), HD)     # d' = h*HD+hd -> h=hh, hd=dd
    src = dd * H + hh
    Wo_p = (Wo[src, :] * WSC).astype(f8)
    # Wv g-major: Wv_v[g*128+p, ko*128+m] = Wv[ko*128+p, g*128+m]
    Wv_v = np.ascontiguousarray(
        (Wv * WSC).reshape(KO, 128, H, 128).transpose(2, 1, 0, 3).reshape(D, D)
    ).astype(f8)
    # block-diag mask, 1/MSC on (r,c) where r%8 == c%8; two copies wide
    r = np.arange(128)
    m = (r[:, None] % SBLK == r[None, :] % SBLK).astype(np.float32) / MSC
    m2 = np.concatenate([m, m], axis=1)

    affine = bool(np.any(bq) or np.any(bk) or np.any(bo)
                  or np.any(gamma != 1.0) or np.any(beta))
    shared = {
        "Wq": _pack_w(Wq.astype(bf)),
        "Wk": _pack_w((Wk * WSC).astype(f8)),
        "Wv": Wv_v,
        "Wo": _pack_w(Wo_p),
        "bv": np.ascontiguousarray(
            (bv * WSC).reshape(KO, 128).T).astype(np.float32),
        "ident": np.eye(128, dtype=bf),
        "identf8": np.eye(128, dtype=f8),
        "mask": m2.astype(bf),
        "ones": np.ones((1, 128), dtype=bf),
    }
    if affine:
        shared.update({
            "bq": (bq * QSC).reshape(1, D).astype(bf),
            "bk": (bk * WSC).reshape(1, D).astype(bf),
            "bo": (bo * QSC).reshape(1, D).astype(bf),
            "gamma": gamma.reshape(1, D).astype(bf),
            "beta": beta.reshape(1, D).astype(bf),
        })
    return qT, kT, vT, shared, affine


def kernel(q, k, v, Wq, bq, Wk, bk, Wv, bv, Wo, bo, gamma, beta, _bl=None,
           _ncores=None, _trace=False):
    ncores = _ncores or NCORES
    bl = _bl or (q.shape[0] // ncores)
    qT, kT, vT, shared, affine = _prep_host_inputs(
        q, k, v, Wq, bq, Wk, bk, Wv, bv, Wo, bo, gamma, beta)
    nc = build_bass(bl, affine=affine)
    in_maps = []
    for c in range(ncores):
        m = dict(shared)
        s = slice(c * bl, (c + 1) * bl)
        m["qT"] = _pack_x(qT[:, s], bl)
        m["kT"] = _pack_x(kT[:, s], bl)
        m["vT"] = _pack_x(vT[:, s], bl)
        in_maps.append(m)
    res = run_bass_kernel_spmd(nc, in_maps, core_ids=list(range(ncores)),
                               trace=_trace)
    outs = [r["out"] for r in res.results]
    full = np.concatenate(outs, axis=0)
    if _trace:
        kernel.last_results = res
    return full.astype(np.float32)


# revision 24
# speedup vs baseline: 1.0317x; 1.0317x over previous
"""MultiHeadAttention (cosine-sim, no softmax) + residual + LayerNorm on 8 TRN2 cores.

Reference math (per sample row x of q/k/v, D=2048, H=16, HD=128):
  qp = q @ Wq + bq   (kept as residual)
  kp = k @ Wk + bk ; vp = v @ Wv + bv
  per head h: qn = qh/||qh||, kn = kh/||kh||
  s[h,g] = (qn_h . kn_g) / HD          # [16,16] per sample
  o[h] = sum_g s[h,g] * vh_g           # [16,128]
  o_flat[hd*16+h] = o[h,hd]            # interleaved flatten
  o2 = o_flat @ Wo + bo
  x = qp + o2 ; out = layernorm(x) * gamma + beta
Sharding: pure data-parallel over batch (4096 rows/core), weights replicated.

Device strategy (per core):
  - The q projection (the residual path, accuracy-critical) runs in bf16.
    The k/v/o projections run in fp8 e4m3 with perf_mode=DoubleRow (2
    contraction rows per PE cell -> half the matmul instructions).  Their
    error feeds only the attention output o2, which is ~0.25% of |x|, so
    fp8's ~3% error contributes ~1e-4 to the final relative error.
  - Scale folding: q is host-scaled by 2^13 and Wk/Wv/Wo by 2^5 so every
    fp8 operand sits near unit RMS; the score mask carries 1/16 so the
    attention output o lands at RMS ~0.6 for its fp8 requantization, and
    the o2 PSUM comes out at exactly 2^13*o2 -- matching the 2^13-scaled
    qp residual with NO descale op (LayerNorm is scale-invariant; eps is
    scaled by 2^26 to stay faithful).  k-side scales cancel in normalize.
  - Per-head norms: one ACT Square [128,512] + one segmented DVE
    tensor_reduce + one ACT Rsqrt per evac (the accum_out path costs a
    334ns ACTIVATION_READ_ACCUMULATOR per head -- 4x more ACT time).
  - PSUM evacuations ride ACT (scalar.copy beats DVE tensor_copy on
    PSUM reads); transpose evacs alternate DVE/ACT to balance.
  - The PE instruction stream is software-pipelined: each n-group's
    normalize+transpose work is emitted one group late, attention o
    matmuls one group late, and the NEXT chunk's v projection matmuls
    are interleaved into the attention groups so the PE never idles
    long enough for the HAM activity monitor to re-throttle the clock.
  - Attention evacs batched 2 blocks per op in shared PSUM tiles.
  - NB: DVE tensor_tensor_reduce with in0==in1 crashes TRN2 HW (passes
    CoreSim); norms use the ACT Square + DVE reduce path instead.
"""

from contextlib import ExitStack

import numpy as np
import ml_dtypes

import concourse.bass as bass
import concourse.bacc as bacc
import concourse.mybir as mybir
import concourse.tile as tile
from concourse.bass_utils import run_bass_kernel_spmd

BF16 = mybir.dt.bfloat16
F8 = mybir.dt.float8e4
F32 = mybir.dt.float32
DR = mybir.MatmulPerfMode.DoubleRow
AF = mybir.ActivationFunctionType

B, D, H, HD = 32768, 2048, 16, 128
NCORES = 8
EPS = 1e-5
CHUNK = 512          # samples per chunk (8 chunks per core at BL=4096)
NG = D // 512        # 4 n-chunks of 512 columns
KO = D // 128        # 16 contraction blocks
KD = KO // 2         # 8 DoubleRow contraction pairs
SBLK = 8             # samples per attention block matmul (8*16 = 128)
NBT = CHUNK // 128   # 4 b-tiles per chunk
NBLK = CHUNK // SBLK # 64 attention blocks per chunk
AGRP = 8             # attention blocks per pipelined group

QSC = 8192.0         # host scale on q (2^13)
WSC = 32.0           # host scale on Wk/Wv/Wo (2^5)
MSC = 16.0           # score mask divisor: 1/MSC instead of 1/HD so that
                     # o_psum = (WSC/MSC)*128*o = 256*o  (fp8 sweet spot)
                     # and o2_psum = 256*WSC*o2 = QSC*o2 (matches QSC*qp)


def build_bass(bl, repeat=1, affine=False):
    """Build the per-core Bass program for bl batch rows (bl % CHUNK == 0)."""
    nc = bacc.Bacc()

    nch = bl // CHUNK
    # inputs/weights host-packed so every DMA slab is contiguous per
    # partition row (8-16KB descriptors instead of 512B strided rows)
    qT = nc.dram_tensor("qT", [128, nch, KO, CHUNK], BF16,
                        kind="ExternalInput")
    kT = nc.dram_tensor("kT", [128, nch, KO, CHUNK], F8,
                        kind="ExternalInput")
    vT = nc.dram_tensor("vT", [128, nch, KO, CHUNK], F8,
                        kind="ExternalInput")
    Wq = nc.dram_tensor("Wq", [128, NG, KO, 512], BF16,
                        kind="ExternalInput")
    Wk = nc.dram_tensor("Wk", [128, NG, KO, 512], F8, kind="ExternalInput")
    Wv = nc.dram_tensor("Wv", [D, D], F8, kind="ExternalInput")  # g-major
    Wo = nc.dram_tensor("Wo", [128, NG, KO, 512], F8, kind="ExternalInput")
    bv = nc.dram_tensor("bv", [128, KO], F32, kind="ExternalInput")
    if affine:
        bq = nc.dram_tensor("bq", [1, D], BF16, kind="ExternalInput")
        bk = nc.dram_tensor("bk", [1, D], BF16, kind="ExternalInput")
        bo = nc.dram_tensor("bo", [1, D], BF16, kind="ExternalInput")
        gamma = nc.dram_tensor("gamma", [1, D], BF16, kind="ExternalInput")
        beta = nc.dram_tensor("beta", [1, D], BF16, kind="ExternalInput")
    ident = nc.dram_tensor("ident", [128, 128], BF16, kind="ExternalInput")
    identf8 = nc.dram_tensor("identf8", [128, 128], F8,
                             kind="ExternalInput")
    mask = nc.dram_tensor("mask", [128, 256], BF16, kind="ExternalInput")
    ones = nc.dram_tensor("ones", [1, 128], BF16, kind="ExternalInput")
    out = nc.dram_tensor("out", [bl, D], F32, kind="ExternalOutput")

    nchunks = bl // CHUNK

    with tile.TileContext(nc) as tc, ExitStack() as ctx:
        consts = ctx.enter_context(tc.tile_pool(name="consts", bufs=1))
        q_pool = ctx.enter_context(tc.tile_pool(name="qbuf", bufs=1))
        kv_pool = ctx.enter_context(tc.tile_pool(name="kvbuf", bufs=2))
        ws_pool = ctx.enter_context(
            tc.tile_pool(name="ws", bufs=1 if affine else 2))
        ws8_pool = ctx.enter_context(tc.tile_pool(name="ws8", bufs=2))
        wsv_pool = ctx.enter_context(tc.tile_pool(name="wsv", bufs=4))
        chunk_pool = ctx.enter_context(tc.tile_pool(name="chunkbuf", bufs=1))
        vhT_pool = ctx.enter_context(tc.tile_pool(name="vhTbuf", bufs=2))
        kp_pool = ctx.enter_context(tc.tile_pool(name="kp", bufs=6))
        nrm_pool = ctx.enter_context(tc.tile_pool(name="nrm", bufs=2))
        st_pool = ctx.enter_context(tc.tile_pool(name="stvb", bufs=AGRP))
        scr_pool = ctx.enter_context(tc.tile_pool(name="scr", bufs=2))
        small_pool = ctx.enter_context(tc.tile_pool(name="small", bufs=4))
        out_pool = ctx.enter_context(tc.tile_pool(name="outb", bufs=4))
        proj_psum = ctx.enter_context(
            tc.tile_pool(name="proj_psum", bufs=4, space="PSUM"))
        att_psum = ctx.enter_context(
            tc.tile_pool(name="att_psum", bufs=4, space="PSUM"))

        # ---- constants ----
        ident_sb = consts.tile([128, 128], BF16)
        nc.sync.dma_start(out=ident_sb, in_=ident[:, :])
        ident8_sb = consts.tile([128, 128], F8)
        nc.sync.dma_start(out=ident8_sb, in_=identf8[:, :])
        mask_sb = consts.tile([128, 2, 128], BF16)
        nc.sync.dma_start(
            out=mask_sb, in_=mask[:, :].rearrange("p (b m) -> p b m", b=2))
        ones_sb = consts.tile([1, 128], BF16)
        nc.sync.dma_start(out=ones_sb, in_=ones[:, :])
        bv_sb = consts.tile([128, KO], F32)
        nc.sync.dma_start(out=bv_sb, in_=bv[:, :])
        eps_sb = consts.tile([128, 1], F32)
        nc.vector.memset(eps_sb, EPS * QSC * QSC)
        if affine:
            bq_sb = consts.tile([1, D], BF16)
            nc.sync.dma_start(out=bq_sb, in_=bq[:, :])
            bk_sb = consts.tile([1, D], BF16)
            nc.sync.dma_start(out=bk_sb, in_=bk[:, :])
            bo_sb = consts.tile([1, D], BF16)
            nc.sync.dma_start(out=bo_sb, in_=bo[:, :])
            g_ap = gamma[:, :]
            gamma_sb = consts.tile([128, D], BF16)
            nc.sync.dma_start(
                out=gamma_sb,
                in_=bass.AP(tensor=g_ap.tensor, offset=g_ap.offset,
                            ap=[[0, 128], [1, D]]))
            b_ap = beta[:, :]
            beta_sb = consts.tile([128, D], BF16)
            nc.sync.dma_start(
                out=beta_sb,
                in_=bass.AP(tensor=b_ap.tensor, offset=b_ap.offset,
                            ap=[[0, 128], [1, D]]))


        for _rep in range(repeat):

            # one head-slab of the v projection: 8 DoubleRow matmuls + evac
            def emit_v_group(vT_sb, vhT_sb, g, c):
                ws_v = wsv_pool.tile([128, D], F8, tag="wsv")
                nc.sync.dma_start(out=ws_v,
                                  in_=Wv[g * 128:(g + 1) * 128, :])
                ws_v3 = ws_v[:, :].rearrange("p (ko m) -> p ko m", m=128)
                ps_v = proj_psum.tile([128, CHUNK], F32, tag="pp",
                                      name=f"pv_{_rep}_{c}_{g}")
                for kd in range(KD):
                    nc.tensor.matmul(
                        ps_v,
                        ws_v3[:, 2 * kd:2 * kd + 2, :],
                        vT_sb[:, 2 * kd:2 * kd + 2, :],
                        start=(kd == 0), stop=(kd == KD - 1),
                        perf_mode=DR)
                # add per-partition bias while copying psum->sbuf bf16
                nc.scalar.activation(
                    out=vhT_sb[:, :, g, :],
                    in_=ps_v[:, :].rearrange("p (blk s) -> p blk s", s=SBLK),
                    func=AF.Identity, bias=bv_sb[:, g:g + 1], scale=1.0)

            vT_next = kv_pool.tile([128, KO, CHUNK], F8, tag="kv",
                                   name=f"vT_{_rep}_0")
            nc.sync.dma_start(out=vT_next, in_=vT[:, 0])
            vhT_cur = None

            for c in range(nchunks):
              b0 = c * CHUNK
              vT_sb = vT_next
              if c == 0:
                  vhT_cur = vhT_pool.tile([128, NBLK, H, SBLK], F8,
                                          tag="vhT", name=f"vhT_{_rep}_0")
                  for g in range(H):
                      emit_v_group(vT_sb, vhT_cur, g, c)
              vhT_sb = vhT_cur
              qT_sb = q_pool.tile([128, KO, CHUNK], BF16, tag="qT")
              nc.sync.dma_start(out=qT_sb, in_=qT[:, c])

              # chunk-lifetime buffers (interleaved [hd,blk,h,s]; b=blk*8+s)
              qp_sb = chunk_pool.tile([128, NBT, D], BF16, tag="qp",
                                      bufs=2)
              qnT_sb = chunk_pool.tile([128, NBLK, H, SBLK], F8, tag="qnT")
              knT_sb = chunk_pool.tile([128, NBLK, H, SBLK], F8, tag="knT")
              oT_sb = chunk_pool.tile([128, H, CHUNK], F8, tag="oT")

              # ---- q then k projections (natural) + normalize + transpose --
              # PASS1 (matmuls+evac) runs one n-group ahead of PASS2
              # (normalize+transposes) so the PE queue never stalls.
              def normalize_group(ng, evacs, dstT):
                for bt, xp in evacs:
                    scr = scr_pool.tile([128, 512], BF16, tag="scr")
                    nc.scalar.activation(out=scr, in_=xp, func=AF.Square)
                    rr = small_pool.tile([128, 4], F32, tag="rr")
                    nc.vector.tensor_reduce(
                        out=rr,
                        in_=scr[:, :].rearrange("p (h m) -> p h m", m=128),
                        axis=mybir.AxisListType.X, op=mybir.AluOpType.add)
                    nc.scalar.activation(out=rr, in_=rr, func=AF.Sqrt)
                    nc.vector.reciprocal(out=rr, in_=rr)
                    nrm = nrm_pool.tile([128, 512], BF16, tag="nrm")
                    for h4 in range(4):
                        nc.vector.tensor_scalar_mul(
                            out=nrm[:, h4 * 128:(h4 + 1) * 128],
                            in0=xp[:, h4 * 128:(h4 + 1) * 128],
                            scalar1=rr[:, h4:h4 + 1])
                    for h4 in range(4):
                        # normal-mode matmul X.T@I transposes ~2x faster
                        # than transpose_mode (no PE_SBUF latency stall,
                        # counts as HAM-warm activity)
                        tp = att_psum.tile([128, 128], F32, tag="ap")
                        nc.tensor.matmul(
                            tp, nrm[:, h4 * 128:(h4 + 1) * 128], ident_sb,
                            start=True, stop=True)
                        dst = dstT[:, bt * 16:(bt + 1) * 16, ng * 4 + h4, :]
                        src = tp[:, :].rearrange("p (blk s) -> p blk s",
                                                 s=SBLK)
                        if h4 % 2 == 0:
                            nc.vector.tensor_copy(out=dst, in_=src)
                        else:
                            nc.scalar.copy(out=dst, in_=src)

              def proj_qk(xT_sb, W3, b_sb, is_q, tail_filler=None,
                          head_filler=None, defer_final=False):
                dstT = qnT_sb if is_q else knT_sb
                ws_tiles = {}

                def fetch(ng):
                    # one-group weight-slab lookahead; Wq rides the sync
                    # ring, Wk the vector ring (parallel queues)
                    if ng < NG and ng not in ws_tiles:
                        if is_q:
                            t = ws_pool.tile([128, KO, 512], BF16, tag="ws")
                            nc.sync.dma_start(out=t, in_=W3[:, ng])
                        else:
                            t = ws8_pool.tile([128, KO, 512], F8, tag="ws8")
                            nc.sync.dma_start(out=t, in_=W3[:, ng])
                        ws_tiles[ng] = t

                fetch(0)
                fetch(1)
                pending = None
                for ng in range(NG):
                    n0 = ng * 512
                    ws = ws_tiles[ng]
                    all_ps = []
                    for half in range(2):
                        bts = (2 * half, 2 * half + 1)
                        ps_pair = [
                            proj_psum.tile([128, 512], F32, tag="pp",
                                           name=f"pp_{c}_{is_q}_{ng}_{bt}")
                            for bt in bts]
                        if is_q:
                            for ko in range(KO):
                                for i, bt in enumerate(bts):
                                    nc.tensor.matmul(
                                        ps_pair[i],
                                        xT_sb[:, ko, bt * 128:(bt + 1) * 128],
                                        ws[:, ko, :], start=(ko == 0),
                                        stop=(not affine and ko == KO - 1))
                        else:
                            for kd in range(KD):
                                for i, bt in enumerate(bts):
                                    nc.tensor.matmul(
                                        ps_pair[i],
                                        xT_sb[:, 2 * kd:2 * kd + 2,
                                              bt * 128:(bt + 1) * 128],
                                        ws[:, 2 * kd:2 * kd + 2, :],
                                        start=(kd == 0),
                                        stop=(not affine and kd == KD - 1),
                                        perf_mode=DR)
                        for i, bt in enumerate(bts):
                            if affine:
                                nc.tensor.matmul(ps_pair[i], ones_sb,
                                                 b_sb[:, n0:n0 + 512],
                                                 start=False, stop=True)
                            all_ps.append((bt, ps_pair[i]))
                    fetch(ng + 1)
                    # the caller's deferred chain (q's last normalize) goes
                    # in front so it overlaps our first two matmul groups
                    if ng == 1 and head_filler is not None:
                        head_filler()
                    # normalize of the PREVIOUS group goes on the engine
                    # queues BEFORE this group's evacs: its Square/mul chain
                    # is ready now, while these evacs must wait for the
                    # matmuls above -- order swapped = ACT head-of-line block
                    if pending is not None:
                        normalize_group(*pending)
                    evacs = []
                    for bt, ps in all_ps:
                        if is_q:
                            xp = qp_sb[:, bt, n0:n0 + 512]
                        else:
                            kp = kp_pool.tile([128, 512], BF16, tag="kp")
                            xp = kp[:, :]
                        nc.scalar.copy(out=xp, in_=ps)
                        evacs.append((bt, xp))
                    pending = (ng, evacs, dstT)
                if defer_final:
                    # the final group's normalize is emitted by the caller
                    # inside the NEXT phase's matmul stream (the in-order PE
                    # queue would otherwise stall on its transposes)
                    final = pending
                    return lambda: normalize_group(*final)
                if tail_filler is not None:
                    tail_filler()
                normalize_group(*pending)
                return None

              kT_sb = kv_pool.tile([128, KO, CHUNK], F8, tag="kv",
                                 name=f"kT_{c}")
              nc.sync.dma_start(out=kT_sb, in_=kT[:, c])
              proj_qk(qT_sb, Wq, bq_sb if affine else None, True)

              # next chunk's v inputs; the first 4 of its 16 projection
              # head-slabs are emitted as the k-projection tail filler (they
              # keep the PE fed while the last k normalize chain drains),
              # the remaining 12 are interleaved into the attention groups
              if c + 1 < nchunks:
                  vT_next = kv_pool.tile([128, KO, CHUNK], F8, tag="kv",
                                         name=f"vT_{_rep}_{c + 1}")
                  nc.sync.dma_start(out=vT_next, in_=vT[:, c + 1])
                  vhT_next = vhT_pool.tile([128, NBLK, H, SBLK], F8,
                                           tag="vhT",
                                           name=f"vhT_{_rep}_{c + 1}")

                  def k_tail():
                      for g in range(4):
                          emit_v_group(vT_next, vhT_next, g, c + 1)
              else:
                  vhT_next = None
                  k_tail = None

              proj_qk(kT_sb, Wk, bk_sb if affine else None, False,
                      tail_filler=k_tail)

              # prefetch the first two Wo slabs (vector ring) so the output
              # projection never waits on weight DMA; they reuse ws8 slots
              # whose Wk readers completed during the k projection
              wo_tiles = {}

              def fetch_wo(ng):
                  if ng < NG and ng not in wo_tiles:
                      t = ws8_pool.tile([128, KO, 512], F8, tag="ws8")
                      nc.sync.dma_start(out=t, in_=Wo[:, ng])
                      wo_tiles[ng] = t

              fetch_wo(0)
              fetch_wo(1)

              # ---- attention: scores + o, 2x8 samples per psum tile ----
              def att_scores(grp):
                outs = []
                for half in range(AGRP // 2):
                    blk0 = grp * AGRP + 2 * half
                    st_ps = att_psum.tile([128, 2, 128], F32, tag="ap")
                    vb_ps = att_psum.tile([128, 2, 128], F32, tag="ap")
                    for j in range(2):
                        nc.tensor.matmul(
                            st_ps[:, j],
                            knT_sb[:, blk0 + j].rearrange("p h s -> p (h s)"),
                            qnT_sb[:, blk0 + j].rearrange("p h s -> p (h s)"),
                            start=True, stop=True)
                        nc.tensor.matmul(
                            vb_ps[:, j],
                            vhT_sb[:, blk0 + j].rearrange("p h s -> p (h s)"),
                            ident8_sb, start=True, stop=True)
                    st_sb = st_pool.tile([128, 2, 128], BF16, tag="st")
                    nc.vector.tensor_mul(out=st_sb, in0=st_ps, in1=mask_sb)
                    vb_sb = st_pool.tile([128, 2, 128], BF16, tag="vb")
                    nc.scalar.copy(out=vb_sb, in_=vb_ps)
                    outs.append((blk0, st_sb, vb_sb))
                return outs

              def att_o(outs):
                for blk0, st_sb, vb_sb in outs:
                    o_ps = att_psum.tile([128, 2, 128], F32, tag="ap")
                    for j in range(2):
                        nc.tensor.matmul(o_ps[:, j], vb_sb[:, j],
                                         st_sb[:, j], start=True, stop=True)
                    s0 = blk0 * SBLK
                    nc.scalar.copy(
                        out=oT_sb[:, :, s0:s0 + 2 * SBLK].rearrange(
                            "p h (b s) -> p b h s", b=2),
                        in_=o_ps[:, :, :].rearrange("p b (h s) -> p b h s",
                                                    h=H))

              prev = None
              for grp in range(NBLK // AGRP):
                cur = att_scores(grp)
                if vhT_next is not None and grp < 6:
                    emit_v_group(vT_next, vhT_next, 4 + 2 * grp, c + 1)
                    emit_v_group(vT_next, vhT_next, 5 + 2 * grp, c + 1)
                if prev is not None:
                    att_o(prev)
                prev = cur
              att_o(prev)
              vhT_cur = vhT_next

              # ---- output projection + residual ----
              for ng in range(NG):
                n0 = ng * 512
                wo_s = wo_tiles[ng]
                for half in range(2):
                    bts = (2 * half, 2 * half + 1)
                    ps_pair = [proj_psum.tile([128, 512], F32, tag="pp",
                                              name=f"po_{c}_{ng}_{half}_{i}")
                               for i in range(2)]
                    for hj in range(KD):
                        for i, bt in enumerate(bts):
                            nc.tensor.matmul(
                                ps_pair[i],
                                oT_sb[:, 2 * hj:2 * hj + 2,
                                      bt * 128:(bt + 1) * 128],
                                wo_s[:, 2 * hj:2 * hj + 2, :],
                                start=(hj == 0),
                                stop=(not affine and hj == KD - 1),
                                perf_mode=DR)
                    for i, bt in enumerate(bts):
                        ps = ps_pair[i]
                        if affine:
                            nc.tensor.matmul(ps, ones_sb,
                                             bo_sb[:, n0:n0 + 512],
                                             start=False, stop=True)
                        # x = qp + o2 (in place into qp_sb)
                        nc.vector.tensor_add(
                            out=qp_sb[:, bt, n0:n0 + 512],
                            in0=qp_sb[:, bt, n0:n0 + 512], in1=ps)
                fetch_wo(ng + 2)

              # ---- layernorm + store ----
              for bt in range(NBT):
                x_ap = qp_sb[:, bt, :]
                stats = small_pool.tile([128, 4, 6], F32, tag="bn")
                for sg in range(4):
                    nc.vector.bn_stats(out=stats[:, sg, :],
                                       in_=x_ap[:, sg * 512:(sg + 1) * 512])
                mv = small_pool.tile([128, 2], F32, tag="mv")
                nc.vector.bn_aggr(out=mv, in_=stats)
                rstd = small_pool.tile([128, 1], F32, tag="rstd")
                nc.scalar.activation(out=rstd, in_=mv[:, 1:2],
                                     func=AF.Sqrt, bias=eps_sb, scale=1.0)
                nc.vector.reciprocal(out=rstd, in_=rstd)
                for ng in range(NG):
                    n0 = ng * 512
                    ot = out_pool.tile([128, 512], F32, tag="ot")
                    # (x - mu) * rstd
                    nc.vector.tensor_scalar(
                        out=ot, in0=x_ap[:, n0:n0 + 512],
                        scalar1=mv[:, 0:1], scalar2=rstd,
                        op0=mybir.AluOpType.subtract,
                        op1=mybir.AluOpType.mult)
                    if affine:
                        nc.vector.tensor_mul(out=ot, in0=ot,
                                             in1=gamma_sb[:, n0:n0 + 512])
                        nc.gpsimd.tensor_add(out=ot, in0=ot,
                                             in1=beta_sb[:, n0:n0 + 512])
                    # SWDGE queue: keeps LN-gated stores off the HWDGE
                    # rings so they never block evacs or input loads
                    nc.gpsimd.dma_start(
                        out=out[b0 + bt * 128:b0 + (bt + 1) * 128,
                                n0:n0 + 512],
                        in_=ot)

    nc.compile()
    return nc


def _pack_w(W):
    # [D, D] -> [128, NG, KO, 512] with W_h[p, ng, ko, m] = W[ko*128+p, ...]
    return np.ascontiguousarray(
        W.reshape(KO, 128, NG, 512).transpose(1, 2, 0, 3))


def _pack_x(xT, bl):
    # [D, bl] -> [128, nch, KO, CHUNK]; xT_h[p, c, ko, s] = xT[ko*128+p, ...]
    return np.ascontiguousarray(
        xT.reshape(KO, 128, bl // CHUNK, CHUNK).transpose(1, 2, 0, 3))


def _prep_host_inputs(q, k, v, Wq, bq, Wk, bk, Wv, bv, Wo, bo, gamma, beta):
    bf = ml_dtypes.bfloat16
    f8 = ml_dtypes.float8_e4m3
    qT = (q.T * QSC).astype(bf)
    kT = k.T.astype(f8)
    vT = v.T.astype(f8)
    # Wo' row h*128+hd  <- Wo row hd*16+h
    hh, dd = np.divmod(np.arange(D), HD)     # d' = h*HD+hd -> h=hh, hd=dd
    src = dd * H + hh
    Wo_p = (Wo[src, :] * WSC).astype(f8)
    # Wv g-major: Wv_v[g*128+p, ko*128+m] = Wv[ko*128+p, g*128+m]
    Wv_v = np.ascontiguousarray(
        (Wv * WSC).reshape(KO, 128, H, 128).transpose(2, 1, 0, 3).reshape(D, D)
    ).astype(f8)
    # block-diag mask, 1/MSC on (r,c) where r%8 == c%8; two copies wide
    r = np.arange(128)
    m = (r[:, None] % SBLK == r[None, :] % SBLK).astype(np.float32) / MSC
    m2 = np.concatenate([m, m], axis=1)

    affine = bool(np.any(bq) or np.any(bk) or np.any(bo)
                  or np.any(gamma != 1.0) or np.any(beta))
    shared = {
        "Wq": _pack_w(Wq.astype(bf)),
        "Wk": _pack_w((Wk * WSC).astype(f8)),
        "Wv": Wv_v,
        "Wo": _pack_w(Wo_p),
        "bv": np.ascontiguousarray(
            (bv * WSC).reshape(KO, 128).T).astype(np.float32),
        "ident": np.eye(128, dtype=bf),
        "identf8": np.eye(128, dtype=f8),
        "mask": m2.astype(bf),
        "ones": np.ones((1, 128), dtype=bf),
    }
    if affine:
        shared.update({
            "bq": (bq * QSC).reshape(1, D).astype(bf),
            "bk": (bk * WSC).reshape(1, D).astype(bf),
            "bo": (bo * QSC).reshape(1, D).astype(bf),
            "gamma": gamma.reshape(1, D).astype(bf),
            "beta": beta.reshape(1, D).astype(bf),
        })
    return qT, kT, vT, shared, affine


def kernel(q, k, v, Wq, bq, Wk, bk, Wv, bv, Wo, bo, gamma, beta, _bl=None,
           _ncores=None, _trace=False):
    ncores = _ncores or NCORES
    bl = _bl or (q.shape[0] // ncores)
    qT, kT, vT, shared, affine = _prep_host_inputs(
        q, k, v, Wq, bq, Wk, bk, Wv, bv, Wo, bo, gamma, beta)
    nc = build_bass(bl, affine=affine)
    in_maps = []
    for c in range(ncores):
        m = dict(shared)
        s = slice(c * bl, (c + 1) * bl)
        m["qT"] = _pack_x(qT[:, s], bl)
        m["kT"] = _pack_x(kT[:, s], bl)
        m["vT"] = _pack_x(vT[:, s], bl)
        in_maps.append(m)
    res = run_bass_kernel_spmd(nc, in_maps, core_ids=list(range(ncores)),
                               trace=_trace)
    outs = [r["out"] for r in res.results]
    full = np.concatenate(outs, axis=0)
    if _trace:
        kernel.last_results = res
    return full.astype(np.float32)
